# revision 31
# baseline (speedup 1.0000x reference)
"""Trainium2 Bass kernel for BiLinearInteractionLayer.

Computes, for every field pair p=(i,j), i<j, of F=32 fields:
    y[b, p, :] = (x[b, i, :] @ W[p].T) * x[b, j, :]
x: [4096, 32, 64] f32, W: [496, 64, 64] f32 -> y: [4096, 496, 64] f32.

Sharding: data-parallel over the batch dim across 8 NeuronCores (512
rows each); the weight stack is replicated.

The kernel is HBM-bound: the dominant cost is writing the 520 MB output.
All device I/O therefore runs in fp16 (inputs rounded host-side, output
converted back host-side), which keeps max error ~1e-3 of output scale
(gate is 2e-2) and halves every stream: per-core traffic drops from
~81 MB (f32/f32r) to ~40 MB.

Per-core algorithm (batch tile of 128 rows at a time):
  - Host pre-transposes layouts (free): the contraction dim d lands on
    SBUF partitions with clean contiguous DMAs, no on-device transposes.
  - For each first-field i, the pairs (i, i+1..31) are contiguous both in
    the pair axis and in the transposed weight columns: one stationary
    xT_i [64d, 128b] serves matmuls streaming W^T columns (N<=512 per
    PSUM bank) into a 4-bank PSUM group [128, (31-i)*64].
  - Even fields live on SBUF partitions 0-63 (PE row group 0), odd
    fields on 64-127 (row group 2): the two K=64 matmul streams execute
    on disjoint halves of the PE array and overlap.
  - The xj factors of a run are x[b, (i+1)*64 : 32*64] -- one contiguous
    slice.
  - Multiply stage is split across two engines so neither becomes the
    bottleneck once DMA drops to ~113 us: a DVE tensor_tensor with a
    PSUM operand runs in 1x mode (~1 elem/cycle/lane @0.96 GHz), but a
    16-bit SBUF-only tensor_tensor runs in 2x mode. So the big runs
    (i <= split_i) are first cast-copied PSUM->SBUF fp16 by the Scalar
    engine (1 elem/cycle/lane @1.2 GHz, sits next to PSUM), then
    multiplied on DVE at 2x; the small tail runs go straight from PSUM
    on DVE at 1x. Both engines land at ~90 us/core, under the DMA wall.
  - One output DMA per (tile, i): 128 rows x (31-i)*128 B.
"""

import itertools

import numpy as np

import concourse.bass as bass
import concourse.mybir as mybir
import concourse.tile as _tile
from concourse.bass_utils import run_bass_kernel_spmd
from concourse.tile import TileContext
from concourse.tile_scheduler import N_PROCS
from concourse.vector_clock import ScopedClock, VectorClock

# --------------------------------------------------------------------------
# Tail-drain patch: the staged walrus rejects >1 sync-wait command on a
# TPB_CTRL (Drain) instruction, but the stock Tile tail-drain attaches one
# wait per outstanding sem lane to a single Drain. Replace it with a ladder
# of single-wait SP nops (one per proc lane) followed by a wait-less drain.
# --------------------------------------------------------------------------


def _split_drain_and_barrier(self, tick_clock, wait_clock):
    nc = self.nc
    g = tick_clock.global_clock
    for p in range(N_PROCS):
        tick = g.peek_next(p) - 1
        if tick <= 0:
            continue
        pc = VectorClock()
        pc.require_at_least(p, tick)
        w = nc.sync.nop(nofuse=True)
        wait_clock.add_sem_waits(w.ins, ScopedClock({None: pc}))
    nc.sync.drain()
    nc.all_engine_barrier()
    assert self.sems is not None
    popped = nc._tile_sem_poison_stack.pop()
    assert popped is self._sem_poison
    nc.clear_and_free_semaphores(list(self.sems.allocated().values()))
    nc.all_engine_barrier()


_tile.TileContext._drain_and_barrier = _split_drain_and_barrier

_wsplit_counter = [0]


def _legalize_single_wait(nc):
    """Hoist extra sem waits onto preceding same-engine NoOps.

    This walrus build encodes at most ONE sync-wait command per TPB
    instruction; Tile's sem-assignment pass freely attaches several.
    Splitting extras onto immediately-preceding NoOps on the same engine
    preserves program order (engines issue in order), hence semantics."""
    import bass_rust

    for fn in nc.m.functions:
        for blk in fn.blocks:
            insts = list(blk.instructions)
            if not any(
                ins.sync_info is not None and len(ins.sync_info.on_wait) > 1
                for ins in insts
            ):
                continue
            out = []
            for ins in insts:
                si = ins.sync_info
                waits = list(si.on_wait) if si is not None else []
                if len(waits) > 1:
                    for w in waits[:-1]:
                        _wsplit_counter[0] += 1
                        nop = mybir.InstNoOp(
                            name=f"I-wsplit-{_wsplit_counter[0]}", ins=[], outs=[]
                        )
                        nop.engine = ins.engine
                        nop.sync_info = bass_rust.SyncInfo(
                            on_wait=[w], on_update=[]
                        )
                        out.append(nop)
                    si.on_wait = [waits[-1]]
                out.append(ins)
            blk.instructions = out


# --------------------------------------------------------------------------
# Problem constants (hardcoded per contract: kernel.py is self-contained).
# --------------------------------------------------------------------------
B, F, D = 4096, 32, 64
NCORES = 8
BL = B // NCORES          # 512 batch rows per core
PT = 128                  # batch tile = SBUF partition count
TILES = BL // PT          # 4 tiles per core
NPAIR = F * (F - 1) // 2  # 496
# pair index of (i, i+1) within itertools.combinations(range(F), 2) order
IDX0 = [0] * F
for _i in range(1, F):
    IDX0[_i] = IDX0[_i - 1] + (F - _i)
# per-parity column offset of field i's run inside its wt half
POFF = [0] * F
for _i in range(2, F):
    POFF[_i] = POFF[_i - 2] + (F - 1 - (_i - 2)) * D
WT_COLS = max(POFF[30] + 1 * D, POFF[31])  # even half is the larger: 16384
WT_COLS = max(WT_COLS, 16384)

# Output staging blocks: run boundaries chosen so each block is ~8K columns
# (~2 MB per 128-row DMA, well into the efficient DMA-size regime).
BLOCKS = [(0, 3), (4, 8), (9, 14), (15, 30)]  # inclusive i ranges
BLK_OF_I = {}
for _b, (_lo, _hi) in enumerate(BLOCKS):
    for _i in range(_lo, _hi + 1):
        BLK_OF_I[_i] = _b
BLK_C0 = [IDX0[lo] * D for lo, hi in BLOCKS]           # absolute y col base
BLK_COLS = [
    (IDX0[hi] + (F - 1 - hi)) * D - IDX0[lo] * D for lo, hi in BLOCKS
]

F32 = mybir.dt.float32
F16 = mybir.dt.float16

_nc_cache = {}


def _build_bass(mm_dt=F16, out_dt=F16, psum_cols=1024, psum_bufs=2, io_bufs=2,
                out_bufs=2, proj_bufs=2, split_i=15):
    nc = bass.Bass(trn_type="TRN2")
    x_d = nc.dram_tensor("x", [BL, F * D], mm_dt, kind="ExternalInput")
    xt_d = nc.dram_tensor("xt", [PT, TILES * (F // 2) * PT], mm_dt,
                          kind="ExternalInput")
    wt_d = nc.dram_tensor("wt", [PT, WT_COLS], mm_dt, kind="ExternalInput")
    y_d = nc.dram_tensor("y", [BL, NPAIR * D], out_dt, kind="ExternalOutput")

    CB = (F // 2) * PT  # 2048 xt cols per batch tile

    with TileContext(nc) as tc:
        with (
            tc.tile_pool(name="wtp", bufs=1) as wtp,
            tc.tile_pool(name="iop", bufs=1) as iop,
            tc.tile_pool(name="projp", bufs=proj_bufs) as projp,
            tc.tile_pool(name="outp", bufs=out_bufs) as outp,
            tc.tile_pool(name="pp", bufs=psum_bufs, space="PSUM") as pp,
        ):
            # All inputs are frontloaded and stay resident (~8 MB total,
            # fits in SBUF alongside the staging pools): tile-boundary
            # compute never waits on an input DMA stuck in a FIFO ring
            # behind output drains. wt streams on the SP HWDGE ring while
            # x/xt ride the gpsimd SWDGE path concurrently.
            wt_s = wtp.tile([PT, WT_COLS], mm_dt)
            x_tiles, xt_tiles = [], []
            for t in range(TILES):
                x_tiles.append(
                    iop.tile([PT, F * D], mm_dt, tag=f"x{t}", name=f"x{t}")
                )
                xt_tiles.append(
                    iop.tile([PT, CB], mm_dt, tag=f"xt{t}", name=f"xt{t}")
                )
            # first slivers sized so the m=0 matmuls/TTs can start as soon
            # as possible, then bulk loads
            nc.gpsimd.dma_start(out=xt_tiles[0][:, 0:256], in_=xt_d[:, 0:256])
            nc.gpsimd.dma_start(
                out=xt_tiles[0][:, 256:CB], in_=xt_d[:, 256:CB]
            )
            nc.gpsimd.dma_start(
                out=x_tiles[0][:, 0:1088], in_=x_d[0:PT, 0:1088]
            )
            nc.gpsimd.dma_start(
                out=x_tiles[0][:, 1088 : F * D], in_=x_d[0:PT, 1088 : F * D]
            )
            # chunked weight load: matmuls for early fields only depend on
            # their own column range (Tile subtile deps), so compute starts
            # after the first sliver of the weights has landed
            wt_chunks = [512, 512, 1024] + [2048] * 7
            w0 = 0
            for wch in wt_chunks:
                nc.sync.dma_start(
                    out=wt_s[:, w0 : w0 + wch], in_=wt_d[:, w0 : w0 + wch]
                )
                w0 += wch
            assert w0 == WT_COLS
            for t in range(1, TILES):
                nc.gpsimd.dma_start(
                    out=xt_tiles[t], in_=xt_d[:, t * CB : (t + 1) * CB]
                )
                nc.gpsimd.dma_start(
                    out=x_tiles[t], in_=x_d[t * PT : (t + 1) * PT, :]
                )
            for t in range(TILES):
                x_s = x_tiles[t]
                xt_s = xt_tiles[t]
                blk_tiles = [
                    outp.tile(
                        [PT, BLK_COLS[b]], out_dt, tag=f"b{b}",
                        name=f"b{b}_{t}",
                    )
                    for b in range(len(BLOCKS))
                ]
                for m in range(F // 2):
                    # split each live parity's field run into PSUM-group
                    # work items of <= psum_cols columns, then alternate
                    # parities so the two PE row groups interleave
                    per_par = []
                    for par in (0, 1):
                        i = 2 * m + par
                        if i > F - 2:
                            continue
                        ncol = (F - 1 - i) * D
                        groups = []
                        for g0 in range(0, ncol, psum_cols):
                            gcols = min(psum_cols, ncol - g0)
                            groups.append((par, i, g0, gcols))
                        per_par.append(groups)
                    order = [
                        g
                        for pair in itertools.zip_longest(*per_par)
                        for g in pair
                        if g is not None
                    ]
                    run_pj = {}
                    for par, i, g0, gcols in order:
                        lhsT = xt_s[par * D : (par + 1) * D,
                                    m * PT : (m + 1) * PT]
                        off = POFF[i] + g0
                        ps = pp.tile(
                            [PT, psum_cols], F32, tag=f"ps{par}",
                            name=f"ps_{t}_{i}_{g0}",
                        )
                        blk = BLK_OF_I[i]
                        out_s = blk_tiles[blk]
                        o0 = IDX0[i] * D - BLK_C0[blk] + g0
                        for k0 in range(0, gcols, 512):
                            kn = min(512, gcols - k0)
                            nc.tensor.matmul(
                                ps[:, k0 : k0 + kn],
                                lhsT,
                                wt_s[par * D : (par + 1) * D,
                                     off + k0 : off + k0 + kn],
                                start=True,
                                stop=True,
                            )
                        ncol = (F - 1 - i) * D
                        if i <= split_i:
                            # big runs: ScalarE casts each PSUM group f32 ->
                            # SBUF fp16 into a per-run proj tile; after the
                            # run's last group, ONE DVE multiply covers the
                            # whole run in 2x (16-bit SBUF) mode -- half
                            # the DVE op count of per-group multiplies
                            if g0 == 0:
                                run_pj[i] = projp.tile(
                                    [PT, 2 * psum_cols], out_dt,
                                    tag=f"pj{par}", name=f"pj_{t}_{i}",
                                )
                            pj = run_pj[i]
                            nc.scalar.copy(
                                out=pj[:, g0 : g0 + gcols],
                                in_=ps[:, :gcols],
                            )
                            if g0 + gcols == ncol:
                                nc.vector.tensor_mul(
                                    out=out_s[:, o0 - g0 : o0 - g0 + ncol],
                                    in0=pj[:, :ncol],
                                    in1=x_s[:, (i + 1) * D :
                                            (i + 1) * D + ncol],
                                )
                        else:
                            # small tail runs: single DVE op from PSUM (1x)
                            xj = x_s[:, (i + 1) * D + g0 :
                                     (i + 1) * D + g0 + gcols]
                            nc.vector.tensor_mul(
                                out=out_s[:, o0 : o0 + gcols],
                                in0=ps[:, :gcols], in1=xj,
                            )
                    # fire each staging block's single ~2 MB DMA once its
                    # last run is complete. Output DMAs alternate between
                    # the SP HWDGE ring and the gpsimd SWDGE path so
                    # descriptor throughput isn't single-ring-bound. (Only
                    # SP/Activation/gpsimd may initiate DMAs; Activation
                    # would head-of-line-block its cast-copies.)
                    for b, (lo, hi) in enumerate(BLOCKS):
                        if hi == 2 * m or hi == 2 * m + 1:
                            dma_eng = nc.sync if b % 2 == 0 else nc.gpsimd
                            dma_eng.dma_start(
                                out=y_d[t * PT : (t + 1) * PT,
                                        BLK_C0[b] : BLK_C0[b] + BLK_COLS[b]],
                                in_=blk_tiles[b],
                            )
    _legalize_single_wait(nc)
    return nc


def _get_nc(mm_dt, out_dt, psum_cols, psum_bufs, io_bufs, out_bufs, proj_bufs,
            split_i):
    key = (str(mm_dt), str(out_dt), psum_cols, psum_bufs, io_bufs, out_bufs,
           proj_bufs, split_i)
    if key not in _nc_cache:
        _nc_cache[key] = _build_bass(
            mm_dt, out_dt, psum_cols, psum_bufs, io_bufs, out_bufs, proj_bufs,
            split_i
        )
    return _nc_cache[key]


_NP_DT = {str(F16): np.float16, str(F32): np.float32,
          str(mybir.dt.float32r): np.float32,
          str(mybir.dt.bfloat16): np.float32}


def _prep_inputs(x, W, mm_dt=F16):
    np_dt = _NP_DT[str(mm_dt)]
    x = np.asarray(x, dtype=np.float32).astype(np_dt)
    W = np.asarray(W, dtype=np.float32).astype(np_dt)
    # wt2[par*64+d, POFF[i] + (j-i-1)*64 + o] = W[(i,j), o, d]
    wt2 = np.zeros((PT, WT_COLS), dtype=np_dt)
    for i in range(F - 1):
        par = i % 2
        npair = F - 1 - i
        blk = W[IDX0[i] : IDX0[i] + npair]           # [npair, D, D]
        blk = blk.transpose(2, 0, 1).reshape(D, npair * D)
        wt2[par * D : (par + 1) * D, POFF[i] : POFF[i] + npair * D] = blk
    in_maps = []
    for c in range(NCORES):
        xl = x[c * BL : (c + 1) * BL]                      # [512, 32, 64]
        x_in = np.ascontiguousarray(xl.reshape(BL, F * D))
        # xt2[par*64+d, t*2048 + m*128 + b] = xl[t*128+b, 2m+par, d]
        xt2 = np.ascontiguousarray(
            xl.reshape(TILES, PT, F // 2, 2, D).transpose(3, 4, 0, 2, 1)
        ).reshape(PT, TILES * (F // 2) * PT)
        in_maps.append({"x": x_in, "xt": xt2, "wt": wt2})
    return in_maps


def _run(x, W, trace=False, mm_dt=F16, out_dt=F16, psum_cols=1024,
         psum_bufs=2, io_bufs=2, out_bufs=2, proj_bufs=2, split_i=15):
    nc = _get_nc(mm_dt, out_dt, psum_cols, psum_bufs, io_bufs, out_bufs,
                 proj_bufs, split_i)
    in_maps = _prep_inputs(x, W, mm_dt)
    res = run_bass_kernel_spmd(nc, in_maps, core_ids=list(range(NCORES)),
                               trace=trace)
    y = np.concatenate(
        [res.results[c]["y"].reshape(BL, NPAIR, D) for c in range(NCORES)],
        axis=0,
    ).astype(np.float32)
    return y, res


def kernel(x, W):
    y, _ = _run(x, W)
    return y


# revision 35
# speedup vs baseline: 1.0103x; 1.0103x over previous
"""Trainium2 Bass kernel for BiLinearInteractionLayer.

Computes, for every field pair p=(i,j), i<j, of F=32 fields:
    y[b, p, :] = (x[b, i, :] @ W[p].T) * x[b, j, :]
x: [4096, 32, 64] f32, W: [496, 64, 64] f32 -> y: [4096, 496, 64] f32.

Sharding: data-parallel over the batch dim across 8 NeuronCores (512
rows each); the weight stack is replicated.

The kernel is HBM-bound: the dominant cost is writing the 520 MB output.
All device I/O therefore runs in fp16 (inputs rounded host-side, output
converted back host-side), which keeps max error ~1e-3 of output scale
(gate is 2e-2) and halves every stream: per-core traffic drops from
~81 MB (f32/f32r) to ~40 MB.

Per-core algorithm (batch tile of 128 rows at a time):
  - Host pre-transposes layouts (free): the contraction dim d lands on
    SBUF partitions with clean contiguous DMAs, no on-device transposes.
  - For each first-field i, the pairs (i, i+1..31) are contiguous both in
    the pair axis and in the transposed weight columns: one stationary
    xT_i [64d, 128b] serves matmuls streaming W^T columns (N<=512 per
    PSUM bank) into a 4-bank PSUM group [128, (31-i)*64].
  - Even fields live on SBUF partitions 0-63 (PE row group 0), odd
    fields on 64-127 (row group 2): the two K=64 matmul streams execute
    on disjoint halves of the PE array and overlap.
  - The xj factors of a run are x[b, (i+1)*64 : 32*64] -- one contiguous
    slice.
  - Multiply stage is split across two engines so neither becomes the
    bottleneck once DMA drops to ~113 us: a DVE tensor_tensor with a
    PSUM operand runs in 1x mode (~1 elem/cycle/lane @0.96 GHz), but a
    16-bit SBUF-only tensor_tensor runs in 2x mode. So the big runs
    (i <= split_i) are first cast-copied PSUM->SBUF fp16 by the Scalar
    engine (1 elem/cycle/lane @1.2 GHz, sits next to PSUM), then
    multiplied on DVE at 2x; the small tail runs go straight from PSUM
    on DVE at 1x. Both engines land at ~90 us/core, under the DMA wall.
  - One output DMA per (tile, i): 128 rows x (31-i)*128 B.
"""

import itertools

import numpy as np

import concourse.bass as bass
import concourse.mybir as mybir
import concourse.tile as _tile
from concourse.bass_utils import run_bass_kernel_spmd
from concourse.tile import TileContext
from concourse.tile_scheduler import N_PROCS
from concourse.vector_clock import ScopedClock, VectorClock

# --------------------------------------------------------------------------
# Tail-drain patch: the staged walrus rejects >1 sync-wait command on a
# TPB_CTRL (Drain) instruction, but the stock Tile tail-drain attaches one
# wait per outstanding sem lane to a single Drain. Replace it with a ladder
# of single-wait SP nops (one per proc lane) followed by a wait-less drain.
# --------------------------------------------------------------------------


def _split_drain_and_barrier(self, tick_clock, wait_clock):
    nc = self.nc
    g = tick_clock.global_clock
    for p in range(N_PROCS):
        tick = g.peek_next(p) - 1
        if tick <= 0:
            continue
        pc = VectorClock()
        pc.require_at_least(p, tick)
        w = nc.sync.nop(nofuse=True)
        wait_clock.add_sem_waits(w.ins, ScopedClock({None: pc}))
    nc.sync.drain()
    nc.all_engine_barrier()
    assert self.sems is not None
    popped = nc._tile_sem_poison_stack.pop()
    assert popped is self._sem_poison
    nc.clear_and_free_semaphores(list(self.sems.allocated().values()))
    nc.all_engine_barrier()


_tile.TileContext._drain_and_barrier = _split_drain_and_barrier

_wsplit_counter = [0]


def _legalize_single_wait(nc):
    """Hoist extra sem waits onto preceding same-engine NoOps.

    This walrus build encodes at most ONE sync-wait command per TPB
    instruction; Tile's sem-assignment pass freely attaches several.
    Splitting extras onto immediately-preceding NoOps on the same engine
    preserves program order (engines issue in order), hence semantics."""
    import bass_rust

    for fn in nc.m.functions:
        for blk in fn.blocks:
            insts = list(blk.instructions)
            if not any(
                ins.sync_info is not None and len(ins.sync_info.on_wait) > 1
                for ins in insts
            ):
                continue
            out = []
            for ins in insts:
                si = ins.sync_info
                waits = list(si.on_wait) if si is not None else []
                if len(waits) > 1:
                    for w in waits[:-1]:
                        _wsplit_counter[0] += 1
                        nop = mybir.InstNoOp(
                            name=f"I-wsplit-{_wsplit_counter[0]}", ins=[], outs=[]
                        )
                        nop.engine = ins.engine
                        nop.sync_info = bass_rust.SyncInfo(
                            on_wait=[w], on_update=[]
                        )
                        out.append(nop)
                    si.on_wait = [waits[-1]]
                out.append(ins)
            blk.instructions = out


# --------------------------------------------------------------------------
# Problem constants (hardcoded per contract: kernel.py is self-contained).
# --------------------------------------------------------------------------
B, F, D = 4096, 32, 64
NCORES = 8
BL = B // NCORES          # 512 batch rows per core
PT = 128                  # batch tile = SBUF partition count
TILES = BL // PT          # 4 tiles per core
NPAIR = F * (F - 1) // 2  # 496
# pair index of (i, i+1) within itertools.combinations(range(F), 2) order
IDX0 = [0] * F
for _i in range(1, F):
    IDX0[_i] = IDX0[_i - 1] + (F - _i)
# per-parity column offset of field i's run inside its wt half
POFF = [0] * F
for _i in range(2, F):
    POFF[_i] = POFF[_i - 2] + (F - 1 - (_i - 2)) * D
WT_COLS = max(POFF[30] + 1 * D, POFF[31])  # even half is the larger: 16384
WT_COLS = max(WT_COLS, 16384)

# Output staging blocks: run boundaries chosen so each block is ~8K columns
# (~2 MB per 128-row DMA, well into the efficient DMA-size regime).
BLOCKS = [(0, 3), (4, 8), (9, 14), (15, 30)]  # inclusive i ranges
BLK_OF_I = {}
for _b, (_lo, _hi) in enumerate(BLOCKS):
    for _i in range(_lo, _hi + 1):
        BLK_OF_I[_i] = _b
BLK_C0 = [IDX0[lo] * D for lo, hi in BLOCKS]           # absolute y col base
BLK_COLS = [
    (IDX0[hi] + (F - 1 - hi)) * D - IDX0[lo] * D for lo, hi in BLOCKS
]

F32 = mybir.dt.float32
F16 = mybir.dt.float16

_nc_cache = {}


def _build_bass(mm_dt=F16, out_dt=F16, psum_cols=1024, psum_bufs=2, io_bufs=2,
                out_bufs=2, proj_bufs=3, split_i=16):
    nc = bass.Bass(trn_type="TRN2")
    x_d = nc.dram_tensor("x", [BL, F * D], mm_dt, kind="ExternalInput")
    xt_d = nc.dram_tensor("xt", [PT, TILES * (F // 2) * PT], mm_dt,
                          kind="ExternalInput")
    wt_d = nc.dram_tensor("wt", [PT, WT_COLS], mm_dt, kind="ExternalInput")
    y_d = nc.dram_tensor("y", [BL, NPAIR * D], out_dt, kind="ExternalOutput")

    CB = (F // 2) * PT  # 2048 xt cols per batch tile

    with TileContext(nc) as tc:
        with (
            tc.tile_pool(name="wtp", bufs=1) as wtp,
            tc.tile_pool(name="iop", bufs=1) as iop,
            tc.tile_pool(name="projp", bufs=proj_bufs) as projp,
            tc.tile_pool(name="outp", bufs=out_bufs) as outp,
            tc.tile_pool(name="pp", bufs=psum_bufs, space="PSUM") as pp,
        ):
            # All inputs are frontloaded and stay resident (~8 MB total,
            # fits in SBUF alongside the staging pools): tile-boundary
            # compute never waits on an input DMA stuck in a FIFO ring
            # behind output drains. wt streams on the SP HWDGE ring while
            # x/xt ride the gpsimd SWDGE path concurrently.
            wt_s = wtp.tile([PT, WT_COLS], mm_dt)
            x_tiles, xt_tiles = [], []
            for t in range(TILES):
                x_tiles.append(
                    iop.tile([PT, F * D], mm_dt, tag=f"x{t}", name=f"x{t}")
                )
                xt_tiles.append(
                    iop.tile([PT, CB], mm_dt, tag=f"xt{t}", name=f"xt{t}")
                )
            # first slivers sized so the m=0 matmuls/TTs can start as soon
            # as possible, then bulk loads
            nc.gpsimd.dma_start(out=xt_tiles[0][:, 0:256], in_=xt_d[:, 0:256])
            nc.gpsimd.dma_start(
                out=xt_tiles[0][:, 256:CB], in_=xt_d[:, 256:CB]
            )
            nc.gpsimd.dma_start(
                out=x_tiles[0][:, 0:1088], in_=x_d[0:PT, 0:1088]
            )
            nc.gpsimd.dma_start(
                out=x_tiles[0][:, 1088 : F * D], in_=x_d[0:PT, 1088 : F * D]
            )
            # chunked weight load: matmuls for early fields only depend on
            # their own column range (Tile subtile deps), so compute starts
            # after the first sliver of the weights has landed
            wt_chunks = [512, 512, 1024] + [2048] * 7
            w0 = 0
            for wch in wt_chunks:
                nc.sync.dma_start(
                    out=wt_s[:, w0 : w0 + wch], in_=wt_d[:, w0 : w0 + wch]
                )
                w0 += wch
            assert w0 == WT_COLS
            for t in range(1, TILES):
                nc.gpsimd.dma_start(
                    out=xt_tiles[t], in_=xt_d[:, t * CB : (t + 1) * CB]
                )
                nc.gpsimd.dma_start(
                    out=x_tiles[t], in_=x_d[t * PT : (t + 1) * PT, :]
                )
            for t in range(TILES):
                x_s = x_tiles[t]
                xt_s = xt_tiles[t]
                blk_tiles = [
                    outp.tile(
                        [PT, BLK_COLS[b]], out_dt, tag=f"b{b}",
                        name=f"b{b}_{t}",
                    )
                    for b in range(len(BLOCKS))
                ]
                for m in range(F // 2):
                    # split each live parity's field run into PSUM-group
                    # work items of <= psum_cols columns, then alternate
                    # parities so the two PE row groups interleave
                    per_par = []
                    for par in (0, 1):
                        i = 2 * m + par
                        if i > F - 2:
                            continue
                        ncol = (F - 1 - i) * D
                        groups = []
                        for g0 in range(0, ncol, psum_cols):
                            gcols = min(psum_cols, ncol - g0)
                            groups.append((par, i, g0, gcols))
                        per_par.append(groups)
                    order = [
                        g
                        for pair in itertools.zip_longest(*per_par)
                        for g in pair
                        if g is not None
                    ]
                    for par, i, g0, gcols in order:
                        lhsT = xt_s[par * D : (par + 1) * D,
                                    m * PT : (m + 1) * PT]
                        off = POFF[i] + g0
                        ps = pp.tile(
                            [PT, psum_cols], F32, tag=f"ps{par}",
                            name=f"ps_{t}_{i}_{g0}",
                        )
                        blk = BLK_OF_I[i]
                        out_s = blk_tiles[blk]
                        o0 = IDX0[i] * D - BLK_C0[blk] + g0
                        for k0 in range(0, gcols, 512):
                            kn = min(512, gcols - k0)
                            nc.tensor.matmul(
                                ps[:, k0 : k0 + kn],
                                lhsT,
                                wt_s[par * D : (par + 1) * D,
                                     off + k0 : off + k0 + kn],
                                start=True,
                                stop=True,
                            )
                        xj = x_s[:, (i + 1) * D + g0 :
                                 (i + 1) * D + g0 + gcols]
                        dst = out_s[:, o0 : o0 + gcols]
                        if i <= split_i:
                            # big runs: ScalarE casts PSUM f32 -> SBUF fp16,
                            # DVE then multiplies in 2x (16-bit SBUF) mode
                            pj = projp.tile(
                                [PT, psum_cols], out_dt, tag=f"pj{par}",
                                name=f"pj_{t}_{i}_{g0}",
                            )
                            nc.scalar.copy(
                                out=pj[:, :gcols], in_=ps[:, :gcols]
                            )
                            nc.vector.tensor_mul(
                                out=dst, in0=pj[:, :gcols], in1=xj
                            )
                        else:
                            # small tail runs: single DVE op from PSUM (1x)
                            nc.vector.tensor_mul(
                                out=dst, in0=ps[:, :gcols], in1=xj
                            )
                    # fire each staging block's single ~2 MB DMA once its
                    # last run is complete. Output DMAs alternate between
                    # the SP HWDGE ring and the gpsimd SWDGE path so
                    # descriptor throughput isn't single-ring-bound. (Only
                    # SP/Activation/gpsimd may initiate DMAs; Activation
                    # would head-of-line-block its cast-copies.)
                    for b, (lo, hi) in enumerate(BLOCKS):
                        if hi == 2 * m or hi == 2 * m + 1:
                            dma_eng = nc.sync if b % 2 == 0 else nc.gpsimd
                            dma_eng.dma_start(
                                out=y_d[t * PT : (t + 1) * PT,
                                        BLK_C0[b] : BLK_C0[b] + BLK_COLS[b]],
                                in_=blk_tiles[b],
                            )
    _legalize_single_wait(nc)
    return nc


def _get_nc(mm_dt, out_dt, psum_cols, psum_bufs, io_bufs, out_bufs, proj_bufs,
            split_i):
    key = (str(mm_dt), str(out_dt), psum_cols, psum_bufs, io_bufs, out_bufs,
           proj_bufs, split_i)
    if key not in _nc_cache:
        _nc_cache[key] = _build_bass(
            mm_dt, out_dt, psum_cols, psum_bufs, io_bufs, out_bufs, proj_bufs,
            split_i
        )
    return _nc_cache[key]


_NP_DT = {str(F16): np.float16, str(F32): np.float32,
          str(mybir.dt.float32r): np.float32,
          str(mybir.dt.bfloat16): np.float32}


def _prep_inputs(x, W, mm_dt=F16):
    np_dt = _NP_DT[str(mm_dt)]
    x = np.asarray(x, dtype=np.float32).astype(np_dt)
    W = np.asarray(W, dtype=np.float32).astype(np_dt)
    # wt2[par*64+d, POFF[i] + (j-i-1)*64 + o] = W[(i,j), o, d]
    wt2 = np.zeros((PT, WT_COLS), dtype=np_dt)
    for i in range(F - 1):
        par = i % 2
        npair = F - 1 - i
        blk = W[IDX0[i] : IDX0[i] + npair]           # [npair, D, D]
        blk = blk.transpose(2, 0, 1).reshape(D, npair * D)
        wt2[par * D : (par + 1) * D, POFF[i] : POFF[i] + npair * D] = blk
    in_maps = []
    for c in range(NCORES):
        xl = x[c * BL : (c + 1) * BL]                      # [512, 32, 64]
        x_in = np.ascontiguousarray(xl.reshape(BL, F * D))
        # xt2[par*64+d, t*2048 + m*128 + b] = xl[t*128+b, 2m+par, d]
        xt2 = np.ascontiguousarray(
            xl.reshape(TILES, PT, F // 2, 2, D).transpose(3, 4, 0, 2, 1)
        ).reshape(PT, TILES * (F // 2) * PT)
        in_maps.append({"x": x_in, "xt": xt2, "wt": wt2})
    return in_maps


def _run(x, W, trace=False, mm_dt=F16, out_dt=F16, psum_cols=1024,
         psum_bufs=2, io_bufs=2, out_bufs=2, proj_bufs=3, split_i=16):
    nc = _get_nc(mm_dt, out_dt, psum_cols, psum_bufs, io_bufs, out_bufs,
                 proj_bufs, split_i)
    in_maps = _prep_inputs(x, W, mm_dt)
    res = run_bass_kernel_spmd(nc, in_maps, core_ids=list(range(NCORES)),
                               trace=trace)
    y = np.concatenate(
        [res.results[c]["y"].reshape(BL, NPAIR, D) for c in range(NCORES)],
        axis=0,
    ).astype(np.float32)
    return y, res


def kernel(x, W):
    y, _ = _run(x, W)
    return y


# revision 36
# speedup vs baseline: 1.0350x; 1.0244x over previous
"""Trainium2 Bass kernel for BiLinearInteractionLayer.

Computes, for every field pair p=(i,j), i<j, of F=32 fields:
    y[b, p, :] = (x[b, i, :] @ W[p].T) * x[b, j, :]
x: [4096, 32, 64] f32, W: [496, 64, 64] f32 -> y: [4096, 496, 64] f32.

Sharding: data-parallel over the batch dim across 8 NeuronCores (512
rows each); the weight stack is replicated.

The kernel is HBM-bound: the dominant cost is writing the 520 MB output.
All device I/O therefore runs in fp16 (inputs rounded host-side, output
converted back host-side), which keeps max error ~1e-3 of output scale
(gate is 2e-2) and halves every stream: per-core traffic drops from
~81 MB (f32/f32r) to ~40 MB.

Per-core algorithm (batch tile of 128 rows at a time):
  - Host pre-transposes layouts (free): the contraction dim d lands on
    SBUF partitions with clean contiguous DMAs, no on-device transposes.
  - For each first-field i, the pairs (i, i+1..31) are contiguous both in
    the pair axis and in the transposed weight columns: one stationary
    xT_i [64d, 128b] serves matmuls streaming W^T columns (N<=512 per
    PSUM bank) into a 4-bank PSUM group [128, (31-i)*64].
  - Even fields live on SBUF partitions 0-63 (PE row group 0), odd
    fields on 64-127 (row group 2): the two K=64 matmul streams execute
    on disjoint halves of the PE array and overlap.
  - The xj factors of a run are x[b, (i+1)*64 : 32*64] -- one contiguous
    slice.
  - Multiply stage is split across two engines so neither becomes the
    bottleneck once DMA drops to ~113 us: a DVE tensor_tensor with a
    PSUM operand runs in 1x mode (~1 elem/cycle/lane @0.96 GHz), but a
    16-bit SBUF-only tensor_tensor runs in 2x mode. So the big runs
    (i <= split_i) are first cast-copied PSUM->SBUF fp16 by the Scalar
    engine (1 elem/cycle/lane @1.2 GHz, sits next to PSUM), then
    multiplied on DVE at 2x; the small tail runs go straight from PSUM
    on DVE at 1x. Both engines land at ~90 us/core, under the DMA wall.
  - One output DMA per (tile, i): 128 rows x (31-i)*128 B.
"""

import itertools

import numpy as np

import concourse.bass as bass
import concourse.mybir as mybir
import concourse.tile as _tile
from concourse.bass_utils import run_bass_kernel_spmd
from concourse.tile import TileContext
from concourse.tile_scheduler import N_PROCS
from concourse.vector_clock import ScopedClock, VectorClock

# --------------------------------------------------------------------------
# Tail-drain patch: the staged walrus rejects >1 sync-wait command on a
# TPB_CTRL (Drain) instruction, but the stock Tile tail-drain attaches one
# wait per outstanding sem lane to a single Drain. Replace it with a ladder
# of single-wait SP nops (one per proc lane) followed by a wait-less drain.
# --------------------------------------------------------------------------


def _split_drain_and_barrier(self, tick_clock, wait_clock):
    nc = self.nc
    g = tick_clock.global_clock
    for p in range(N_PROCS):
        tick = g.peek_next(p) - 1
        if tick <= 0:
            continue
        pc = VectorClock()
        pc.require_at_least(p, tick)
        w = nc.sync.nop(nofuse=True)
        wait_clock.add_sem_waits(w.ins, ScopedClock({None: pc}))
    nc.sync.drain()
    nc.all_engine_barrier()
    assert self.sems is not None
    popped = nc._tile_sem_poison_stack.pop()
    assert popped is self._sem_poison
    nc.clear_and_free_semaphores(list(self.sems.allocated().values()))
    nc.all_engine_barrier()


_tile.TileContext._drain_and_barrier = _split_drain_and_barrier

_wsplit_counter = [0]


def _legalize_single_wait(nc):
    """Hoist extra sem waits onto preceding same-engine NoOps.

    This walrus build encodes at most ONE sync-wait command per TPB
    instruction; Tile's sem-assignment pass freely attaches several.
    Splitting extras onto immediately-preceding NoOps on the same engine
    preserves program order (engines issue in order), hence semantics."""
    import bass_rust

    for fn in nc.m.functions:
        for blk in fn.blocks:
            insts = list(blk.instructions)
            if not any(
                ins.sync_info is not None and len(ins.sync_info.on_wait) > 1
                for ins in insts
            ):
                continue
            out = []
            for ins in insts:
                si = ins.sync_info
                waits = list(si.on_wait) if si is not None else []
                if len(waits) > 1:
                    for w in waits[:-1]:
                        _wsplit_counter[0] += 1
                        nop = mybir.InstNoOp(
                            name=f"I-wsplit-{_wsplit_counter[0]}", ins=[], outs=[]
                        )
                        nop.engine = ins.engine
                        nop.sync_info = bass_rust.SyncInfo(
                            on_wait=[w], on_update=[]
                        )
                        out.append(nop)
                    si.on_wait = [waits[-1]]
                out.append(ins)
            blk.instructions = out


# --------------------------------------------------------------------------
# Problem constants (hardcoded per contract: kernel.py is self-contained).
# --------------------------------------------------------------------------
B, F, D = 4096, 32, 64
NCORES = 8
BL = B // NCORES          # 512 batch rows per core
PT = 128                  # batch tile = SBUF partition count
TILES = BL // PT          # 4 tiles per core
NPAIR = F * (F - 1) // 2  # 496
# pair index of (i, i+1) within itertools.combinations(range(F), 2) order
IDX0 = [0] * F
for _i in range(1, F):
    IDX0[_i] = IDX0[_i - 1] + (F - _i)
# per-parity column offset of field i's run inside its wt half
POFF = [0] * F
for _i in range(2, F):
    POFF[_i] = POFF[_i - 2] + (F - 1 - (_i - 2)) * D
WT_COLS = max(POFF[30] + 1 * D, POFF[31])  # even half is the larger: 16384
WT_COLS = max(WT_COLS, 16384)

# Output staging blocks: run boundaries chosen so each block DMA is
# ~1.3-1.6 MB (well into the efficient DMA-size regime) with a small last
# block so the end-of-kernel drain is short.
BLOCKS = [(0, 2), (3, 5), (6, 9), (10, 14), (15, 21), (22, 30)]
BLK_OF_I = {}
for _b, (_lo, _hi) in enumerate(BLOCKS):
    for _i in range(_lo, _hi + 1):
        BLK_OF_I[_i] = _b
BLK_C0 = [IDX0[lo] * D for lo, hi in BLOCKS]           # absolute y col base
BLK_COLS = [
    (IDX0[hi] + (F - 1 - hi)) * D - IDX0[lo] * D for lo, hi in BLOCKS
]

F32 = mybir.dt.float32
F16 = mybir.dt.float16

_nc_cache = {}


def _build_bass(mm_dt=F16, out_dt=F16, psum_cols=1024, psum_bufs=2, io_bufs=2,
                out_bufs=2, proj_bufs=3, split_i=16):
    nc = bass.Bass(trn_type="TRN2")
    x_d = nc.dram_tensor("x", [BL, F * D], mm_dt, kind="ExternalInput")
    xt_d = nc.dram_tensor("xt", [PT, TILES * (F // 2) * PT], mm_dt,
                          kind="ExternalInput")
    wt_d = nc.dram_tensor("wt", [PT, WT_COLS], mm_dt, kind="ExternalInput")
    y_d = nc.dram_tensor("y", [BL, NPAIR * D], out_dt, kind="ExternalOutput")

    CB = (F // 2) * PT  # 2048 xt cols per batch tile

    with TileContext(nc) as tc:
        with (
            tc.tile_pool(name="wtp", bufs=1) as wtp,
            tc.tile_pool(name="iop", bufs=1) as iop,
            tc.tile_pool(name="projp", bufs=proj_bufs) as projp,
            tc.tile_pool(name="outp", bufs=out_bufs) as outp,
            tc.tile_pool(name="pp", bufs=psum_bufs, space="PSUM") as pp,
        ):
            # All inputs are frontloaded and stay resident (~8 MB total,
            # fits in SBUF alongside the staging pools): tile-boundary
            # compute never waits on an input DMA stuck in a FIFO ring
            # behind output drains. wt streams on the SP HWDGE ring while
            # x/xt ride the gpsimd SWDGE path concurrently.
            wt_s = wtp.tile([PT, WT_COLS], mm_dt)
            x_tiles, xt_tiles = [], []
            for t in range(TILES):
                x_tiles.append(
                    iop.tile([PT, F * D], mm_dt, tag=f"x{t}", name=f"x{t}")
                )
                xt_tiles.append(
                    iop.tile([PT, CB], mm_dt, tag=f"xt{t}", name=f"xt{t}")
                )
            # first slivers sized so the m=0 matmuls/TTs can start as soon
            # as possible, then bulk loads
            nc.gpsimd.dma_start(out=xt_tiles[0][:, 0:256], in_=xt_d[:, 0:256])
            nc.gpsimd.dma_start(
                out=xt_tiles[0][:, 256:CB], in_=xt_d[:, 256:CB]
            )
            nc.gpsimd.dma_start(
                out=x_tiles[0][:, 0:1088], in_=x_d[0:PT, 0:1088]
            )
            nc.gpsimd.dma_start(
                out=x_tiles[0][:, 1088 : F * D], in_=x_d[0:PT, 1088 : F * D]
            )
            # chunked weight load: matmuls for early fields only depend on
            # their own column range (Tile subtile deps), so compute starts
            # after the first sliver of the weights has landed
            wt_chunks = [512, 512, 1024] + [2048] * 7
            w0 = 0
            for wch in wt_chunks:
                nc.sync.dma_start(
                    out=wt_s[:, w0 : w0 + wch], in_=wt_d[:, w0 : w0 + wch]
                )
                w0 += wch
            assert w0 == WT_COLS
            for t in range(1, TILES):
                nc.gpsimd.dma_start(
                    out=xt_tiles[t], in_=xt_d[:, t * CB : (t + 1) * CB]
                )
                nc.gpsimd.dma_start(
                    out=x_tiles[t], in_=x_d[t * PT : (t + 1) * PT, :]
                )
            for t in range(TILES):
                x_s = x_tiles[t]
                xt_s = xt_tiles[t]
                blk_tiles = [
                    outp.tile(
                        [PT, BLK_COLS[b]], out_dt, tag=f"b{b}",
                        name=f"b{b}_{t}",
                    )
                    for b in range(len(BLOCKS))
                ]
                for m in range(F // 2):
                    # split each live parity's field run into PSUM-group
                    # work items of <= psum_cols columns, then alternate
                    # parities so the two PE row groups interleave
                    per_par = []
                    for par in (0, 1):
                        i = 2 * m + par
                        if i > F - 2:
                            continue
                        ncol = (F - 1 - i) * D
                        groups = []
                        for g0 in range(0, ncol, psum_cols):
                            gcols = min(psum_cols, ncol - g0)
                            groups.append((par, i, g0, gcols))
                        per_par.append(groups)
                    order = [
                        g
                        for pair in itertools.zip_longest(*per_par)
                        for g in pair
                        if g is not None
                    ]
                    for par, i, g0, gcols in order:
                        lhsT = xt_s[par * D : (par + 1) * D,
                                    m * PT : (m + 1) * PT]
                        off = POFF[i] + g0
                        ps = pp.tile(
                            [PT, psum_cols], F32, tag=f"ps{par}",
                            name=f"ps_{t}_{i}_{g0}",
                        )
                        blk = BLK_OF_I[i]
                        out_s = blk_tiles[blk]
                        o0 = IDX0[i] * D - BLK_C0[blk] + g0
                        for k0 in range(0, gcols, 512):
                            kn = min(512, gcols - k0)
                            nc.tensor.matmul(
                                ps[:, k0 : k0 + kn],
                                lhsT,
                                wt_s[par * D : (par + 1) * D,
                                     off + k0 : off + k0 + kn],
                                start=True,
                                stop=True,
                            )
                        xj = x_s[:, (i + 1) * D + g0 :
                                 (i + 1) * D + g0 + gcols]
                        dst = out_s[:, o0 : o0 + gcols]
                        if i <= split_i:
                            # big runs: ScalarE casts PSUM f32 -> SBUF fp16,
                            # DVE then multiplies in 2x (16-bit SBUF) mode
                            pj = projp.tile(
                                [PT, psum_cols], out_dt, tag=f"pj{par}",
                                name=f"pj_{t}_{i}_{g0}",
                            )
                            nc.scalar.copy(
                                out=pj[:, :gcols], in_=ps[:, :gcols]
                            )
                            nc.vector.tensor_mul(
                                out=dst, in0=pj[:, :gcols], in1=xj
                            )
                        else:
                            # small tail runs: single DVE op from PSUM (1x)
                            nc.vector.tensor_mul(
                                out=dst, in0=ps[:, :gcols], in1=xj
                            )
                    # fire each staging block's single ~2 MB DMA once its
                    # last run is complete. Output DMAs alternate between
                    # the SP HWDGE ring and the gpsimd SWDGE path so
                    # descriptor throughput isn't single-ring-bound. (Only
                    # SP/Activation/gpsimd may initiate DMAs; Activation
                    # would head-of-line-block its cast-copies.)
                    for b, (lo, hi) in enumerate(BLOCKS):
                        if hi == 2 * m or hi == 2 * m + 1:
                            dma_eng = nc.sync if b % 2 == 0 else nc.gpsimd
                            dma_eng.dma_start(
                                out=y_d[t * PT : (t + 1) * PT,
                                        BLK_C0[b] : BLK_C0[b] + BLK_COLS[b]],
                                in_=blk_tiles[b],
                            )
    _legalize_single_wait(nc)
    return nc


def _get_nc(mm_dt, out_dt, psum_cols, psum_bufs, io_bufs, out_bufs, proj_bufs,
            split_i):
    key = (str(mm_dt), str(out_dt), psum_cols, psum_bufs, io_bufs, out_bufs,
           proj_bufs, split_i)
    if key not in _nc_cache:
        _nc_cache[key] = _build_bass(
            mm_dt, out_dt, psum_cols, psum_bufs, io_bufs, out_bufs, proj_bufs,
            split_i
        )
    return _nc_cache[key]


_NP_DT = {str(F16): np.float16, str(F32): np.float32,
          str(mybir.dt.float32r): np.float32,
          str(mybir.dt.bfloat16): np.float32}


def _prep_inputs(x, W, mm_dt=F16):
    np_dt = _NP_DT[str(mm_dt)]
    x = np.asarray(x, dtype=np.float32).astype(np_dt)
    W = np.asarray(W, dtype=np.float32).astype(np_dt)
    # wt2[par*64+d, POFF[i] + (j-i-1)*64 + o] = W[(i,j), o, d]
    wt2 = np.zeros((PT, WT_COLS), dtype=np_dt)
    for i in range(F - 1):
        par = i % 2
        npair = F - 1 - i
        blk = W[IDX0[i] : IDX0[i] + npair]           # [npair, D, D]
        blk = blk.transpose(2, 0, 1).reshape(D, npair * D)
        wt2[par * D : (par + 1) * D, POFF[i] : POFF[i] + npair * D] = blk
    in_maps = []
    for c in range(NCORES):
        xl = x[c * BL : (c + 1) * BL]                      # [512, 32, 64]
        x_in = np.ascontiguousarray(xl.reshape(BL, F * D))
        # xt2[par*64+d, t*2048 + m*128 + b] = xl[t*128+b, 2m+par, d]
        xt2 = np.ascontiguousarray(
            xl.reshape(TILES, PT, F // 2, 2, D).transpose(3, 4, 0, 2, 1)
        ).reshape(PT, TILES * (F // 2) * PT)
        in_maps.append({"x": x_in, "xt": xt2, "wt": wt2})
    return in_maps


def _run(x, W, trace=False, mm_dt=F16, out_dt=F16, psum_cols=1024,
         psum_bufs=2, io_bufs=2, out_bufs=2, proj_bufs=3, split_i=16):
    nc = _get_nc(mm_dt, out_dt, psum_cols, psum_bufs, io_bufs, out_bufs,
                 proj_bufs, split_i)
    in_maps = _prep_inputs(x, W, mm_dt)
    res = run_bass_kernel_spmd(nc, in_maps, core_ids=list(range(NCORES)),
                               trace=trace)
    y = np.concatenate(
        [res.results[c]["y"].reshape(BL, NPAIR, D) for c in range(NCORES)],
        axis=0,
    ).astype(np.float32)
    return y, res


def kernel(x, W):
    y, _ = _run(x, W)
    return y
